# revision 1
# baseline (speedup 1.0000x reference)
"""Multi-head self-attention on 8 TRN2 NeuronCores.

Problem: x[2,2048,1024] -> qkv proj -> 16-head attention -> out proj.
Sharding: core c handles batch b=c//4 and head group g=c%4 (4 heads each).
Each core computes a partial output y_c[2048,1024] = attn_out_heads(g) @ W_proj[rows g];
host sums the 4 partials per batch and adds b_proj.

Design notes (from HW measurement, ~224 us/core vs 253 us fp32r baseline):
  - stage 1 (qkv projection) runs DENSE, back-to-back: spreading these
    matmuls thinly into the attention stream measured 1.5-2x slower on HW
    (PE HAM clock-gate drops to 1.2 GHz in sparse phases + cross-engine
    semaphore chains); only the q blocks for i 1024-2047 and the
    out-projection drip into the attention stream, as whole dense blocks.
    Within stage 1, consecutive blocks' 8-matmul accumulation chains are
    emitted with a 4-matmul skew (sequential accumulating chains measured
    ~16% slower than interleaved: 227 vs 192 ns/matmul).
  - input DMAs are few + big (one InstDMACopy splits over all 16 SDMA
    engines; dispatch is ~625ns each) and ordered by first compute use
    (W1 q/k cols + x cols 0-511 first).
  - attention pipeline runs as 512-wide micro-steps (scores -> exp -> PV
    per half-chunk) with a 4-deep ss PSUM rotation (4x 1-bank tiles; same
    banks as 2x 1024-wide): the scores matmul for micro-step n+1 is
    emitted before the PV of step n, and the doubled pipeline depth
    absorbs cross-engine semaphore latency (measured -18% vs the
    1024-wide 2-buf pipeline in a same-window A/B).
  - the last processed head is an even head (its softmax normalize needs
    no partition-shift DMA) and runs as two 512-wide i-passes so the
    final out-projection overlaps its exp stream; the very last projection
    blocks bridge the final normalize to keep PE warm; PSUM evacuations in
    the tail alternate DVE/ACT.
  - layouts as in the earlier fp32r version: x transposed on host, q/k
    produced transposed (qkT[f,s]), k zero-padded to K=128, v natural with
    a ones column per head so the PV matmul also yields softmax denominators.
  - bf16 for x, W1, q, k, v, exp(probs) and the y partials (halves DMA and
    SBUF; errors average out through the softmax weighted sum); fp32r for
    outT/W_proj so the final projection stays accurate. Measured output
    rel-inf error ~7e-3 vs f64 (gate 2e-2).
  - exp [128,512] tiles straight out of PSUM, 1/sqrt(hd) scale fused, no
    max-subtraction (scores bounded for N(0,1)-scale inputs).
"""

import numpy as np

N_CORES = 8
B, S, D = 2, 2048, 1024
H, HD = 16, 64
HPC = 4          # heads per core
F_QK = 512      # q+k features per core (4 heads x 64 x 2)
F_V = 256       # v features per core
FT = 768        # total qkv features per core
SC = 512        # seq chunk (matmul N)
NSC = S // SC   # 4
NJ = S // 128   # 16 j-blocks
NDC = D // 128  # 8 contraction chunks

_CACHE = {}


def _build(repeat=1):
    import contextlib
    import concourse.bass as bass  # noqa: F401
    import concourse.mybir as mybir
    import concourse.tile as tile
    from concourse import bacc

    F32, F32R = mybir.dt.float32, mybir.dt.float32r
    BF16 = mybir.dt.bfloat16

    nc = bacc.Bacc("TRN2", target_bir_lowering=False, num_devices=N_CORES)
    xT = nc.declare_dram_parameter("xT", [D, S], BF16, isOutput=False)
    W1 = nc.declare_dram_parameter("W1", [D, FT], BF16, isOutput=False)
    b1 = nc.declare_dram_parameter("b1", [FT, 1], F32, isOutput=False)
    Wp = nc.declare_dram_parameter("Wp", [HPC * HD, D], F32R, isOutput=False)
    y = nc.declare_dram_parameter("y", [S, D], BF16, isOutput=True)

    with tile.TileContext(nc) as tc:
        with (
            tc.tile_pool(name="weights", bufs=1) as wpool,
            tc.tile_pool(name="persist", bufs=1) as persist,
            tc.tile_pool(name="xin", bufs=1) as xpool,
            tc.tile_pool(name="etile", bufs=6) as epool,
            tc.tile_pool(name="yout", bufs=4) as ypool,
            tc.tile_pool(name="small", bufs=4) as spool,
            tc.tile_pool(name="psA", bufs=2, space="PSUM") as psA,
            tc.tile_pool(name="psS", bufs=2, space="PSUM") as psS,
            tc.tile_pool(name="psO", bufs=2, space="PSUM") as psO,
        ):
            # ---- inputs, ordered by when compute first needs them; chunks
            # of 4 dc-rows go out as single multi-dim-AP DMAs (each
            # InstDMACopy is split across all 16 SDMA engines, and dispatch
            # is ~625ns per instruction, so few + big wins).
            def rows_dma(dst, dst_w, src, r0, nr, src_c0, ncol, dst_c0=None):
                """dst[:, j*dst_w+dst_c0 :+ncol] <-
                src[(r0+j)*128:(r0+j+1)*128, src_c0:src_c0+ncol] per j"""
                if dst_c0 is None:
                    dst_c0 = src_c0
                s = src[r0 * 128:r0 * 128 + 1, 0:1]
                width = src.shape[-1]
                in_ap = bass.AP(tensor=s.tensor, offset=s.offset + src_c0,
                                ap=[[width, 128], [128 * width, nr], [1, ncol]])
                pp = dst.ap[0][0]
                out_ap = bass.AP(tensor=dst.tensor, offset=dst.offset + dst_c0,
                                 ap=[[pp, 128], [dst_w, nr], [1, ncol]])
                nc.sync.dma_start(out=out_ap, in_=in_ap)

            w1b = [wpool.tile([128, 4 * FT], BF16, tag=f"w1b_{g}",
                              name=f"w1b_{g}") for g in range(2)]
            w1t = [w1b[dc // 4][:, (dc % 4) * FT:(dc % 4 + 1) * FT]
                   for dc in range(NDC)]
            xb = [[xpool.tile([128, 4096], BF16, tag=f"xb_{pair}_{g}",
                              name=f"xb_{pair}_{g}") for g in range(2)]
                  for pair in range(2)]
            xts2 = [[xb[pair][dc // 4][:, (dc % 4) * 1024:(dc % 4 + 1) * 1024]
                     for dc in range(NDC)] for pair in range(2)]
            # need-order: W1 q+k cols / x cols 0-511 (h0 i 0-511 + k,v sc0),
            # then W1 v cols, then x cols 512-1023, then x cols 1024-2047.
            rows_dma(w1b[0], FT, W1, 0, 4, 0, 384)
            rows_dma(xb[0][0], 1024, xT, 0, 4, 0, 512)
            rows_dma(w1b[1], FT, W1, 4, 4, 0, 384)
            rows_dma(xb[0][1], 1024, xT, 4, 4, 0, 512)
            rows_dma(w1b[0], FT, W1, 0, 4, 384, 384)
            rows_dma(w1b[1], FT, W1, 4, 4, 384, 384)
            rows_dma(xb[0][0], 1024, xT, 0, 4, 512, 512)
            rows_dma(xb[0][1], 1024, xT, 4, 4, 512, 512)
            b6 = wpool.tile([128, 6], F32, tag="b6", name="b6")
            b1s = b1[0:128, 0:1]
            b6_ap = bass.AP(tensor=b1s.tensor, offset=b1s.offset,
                            ap=[[1, 128], [128, 6]])
            nc.sync.dma_start(out=b6, in_=b6_ap)
            bv = wpool.tile([128, F_V], F32, tag="bv", name="bv")
            bvsrc = b1[F_QK:FT, 0:1]
            bv_ap = bass.AP(tensor=bvsrc.tensor, offset=bvsrc.offset,
                            ap=[[0, 128], [1, F_V]])
            nc.sync.dma_start(out=bv, in_=bv_ap)
            ones = wpool.tile([128, 1], F32, tag="ones", name="ones")
            nc.vector.memset(ones, 1.0)
            # persistent attention tiles whose constant parts (k zero-pad
            # halves, v ones columns) are written once, outside the repeat
            # loop: per-iteration writes only touch the disjoint data parts
            kpad = [wpool.tile([128, S], BF16, tag=f"kpad_{h}",
                               name=f"kpad_{h}") for h in range(HPC)]
            for h in range(HPC):
                zr = slice(64, 128) if h % 2 == 0 else slice(0, 64)
                nc.vector.memset(kpad[h].bitcast(F32)[zr, :], 0.0)
            v4 = [wpool.tile([128, HPC * (HD + 1)], BF16, tag=f"v4_{jc}",
                             name=f"v4_{jc}") for jc in range(NJ)]
            for jc in range(NJ):
                for h in range(HPC):
                    nc.vector.tensor_copy(
                        v4[jc][:, h * (HD + 1) + HD:(h + 1) * (HD + 1)], ones)
            rows_dma(xb[1][0], 1024, xT, 0, 4, 1024, 1024, dst_c0=0)
            rows_dma(xb[1][1], 1024, xT, 4, 4, 1024, 1024, dst_c0=0)
            wpt = []
            for p in range(2):
                t = wpool.tile([128, D], F32R, tag=f"wp_{p}", name=f"wp_{p}")
                nc.sync.dma_start(out=t, in_=Wp[p * 128:(p + 1) * 128, :])
                wpt.append(t)

            if repeat > 1:
                ET = mybir.EngineType
                loop_cm = tc.For_i(0, repeat, 1,
                                   hint_engines=(ET.PE, ET.DVE, ET.Activation,
                                                 ET.Pool, ET.SP))
            else:
                loop_cm = contextlib.nullcontext()
            with loop_cm:
                _emit_body(nc, tc, mybir, locals())
    nc.compile()
    return nc


def _emit_body(nc, tc, mybir, env):
    from collections import deque

    F32, F32R = mybir.dt.float32, mybir.dt.float32r
    BF16 = mybir.dt.bfloat16
    AF = mybir.ActivationFunctionType
    w1t, wpt, b6, bv, ones = (env[k] for k in ("w1t", "wpt", "b6", "bv", "ones"))
    xts2, y = env["xts2"], env["y"]
    wpool, persist, epool, ypool, spool = (
        env[k] for k in ("wpool", "persist", "epool", "ypool", "spool"))
    psA, psS, psO = env["psA"], env["psS"], env["psO"]

    def xts(sc, dc):
        return xts2[sc // 2][dc][:, (sc % 2) * SC:(sc % 2 + 1) * SC]

    # persistent activation tiles
    qk = [persist.tile([128, S], BF16, tag=f"qk_{p}", name=f"qk_{p}")
          for p in range(2)]
    kpad, v4 = env["kpad"], env["v4"]
    outT = [persist.tile([128, S], F32R, tag=f"outT_{p}", name=f"outT_{p}")
            for p in range(2)]

    # ---- stage-1 emitters; drip-queued blocks are split into two 4-dc
    # halves so a single drained step stays ~2048 PE cycles ----
    def emit_q_half(sc, fb, half, state):
        if half == 0:
            state["pq"] = psA.tile([128, SC], F32, tag="mm", name="pq")
        pq = state["pq"]
        for dc in range(4 * half, 4 * half + 4):
            nc.tensor.matmul(pq, w1t[dc][:, fb * 128:(fb + 1) * 128],
                             xts(sc, dc), start=(dc == 0), stop=(dc == NDC - 1))
        if half == 1:
            ssl1 = slice(sc * SC, (sc + 1) * SC)
            if fb < 2:
                nc.vector.tensor_scalar_add(qk[fb][:, ssl1], pq,
                                            b6[:, fb:fb + 1])
            else:
                ke, ko = kpad[2 * (fb - 2)], kpad[2 * (fb - 2) + 1]
                nc.vector.tensor_scalar_add(ke[0:64, ssl1], pq[0:64, :],
                                            b6[0:64, fb:fb + 1])
                nc.vector.tensor_scalar_add(ko[64:128, ssl1], pq[64:128, :],
                                            b6[64:128, fb:fb + 1])

    def emit_qk_block(sc, fb):
        state = {}
        emit_q_half(sc, fb, 0, state)
        emit_q_half(sc, fb, 1, state)

    def emit_v_block(sc, sb):
        jc = sc * 4 + sb
        pv = psA.tile([128, F_V], F32, tag="mm", name="pv")
        for dc in range(NDC):
            nc.tensor.matmul(pv, xts(sc, dc)[:, sb * 128:(sb + 1) * 128],
                             w1t[dc][:, F_QK:FT],
                             start=(dc == 0), stop=(dc == NDC - 1))
        for h in range(HPC):
            nc.vector.tensor_add(v4[jc][:, h * (HD + 1):h * (HD + 1) + HD],
                                 pv[:, h * HD:(h + 1) * HD],
                                 bv[:, h * HD:(h + 1) * HD])

    # ---- out-projection (merged 1024-wide y DMA per s-block) ----
    def make_proj_steps(sblk, tail=False):
        # both output-column halves as one step with the two 2-matmul
        # accumulation chains interleaved (sequential accumulating chains
        # measure ~16% slower than interleaved on HW)
        ssl = slice(sblk * 128, (sblk + 1) * 128)

        def step():
            ysb = ypool.tile([128, 1024], BF16, tag="ysb", name="ysb")
            py0 = psA.tile([128, SC], F32, tag="mm", name="py0")
            py1 = psA.tile([128, SC], F32, tag="mm", name="py1")
            for p in range(2):
                nc.tensor.matmul(py0, outT[p][:, ssl], wpt[p][:, 0:SC],
                                 start=(p == 0), stop=(p == 1))
                nc.tensor.matmul(py1, outT[p][:, ssl], wpt[p][:, SC:1024],
                                 start=(p == 0), stop=(p == 1))
            # tail=True: ACT is idle after the last exp — split the PSUM
            # evacuations between DVE and ACT so neither serializes the tail
            if tail:
                nc.scalar.copy(ysb[:, 0:SC], py0)
            else:
                nc.vector.tensor_copy(ysb[:, 0:SC], py0)
            nc.vector.tensor_copy(ysb[:, SC:1024], py1)
            nc.sync.dma_start(out=y[ssl, :], in_=ysb)
        return [step]

    # ---- drip queue ----
    workq = deque()

    def drain(n=1):
        for _ in range(n):
            if workq:
                workq.popleft()()

    def q_steps(sc, fb):
        # one dense 8-matmul block per step: interleaving finer-grained
        # work into the attention stream measures slower on HW
        return [lambda: emit_qk_block(sc, fb)]

    # STAGE 1, dense and software-pipelined: consecutive blocks' 8-matmul
    # accumulation chains are emitted with a 4-matmul skew so every chain
    # step has a foreign matmul between its own (sequential accumulating
    # chains measured ~16% slower than interleaved on HW), while the
    # 2-slot PSUM rotation keeps covering the DVE evacuations.
    def qk_spec(sc, fb):
        def alloc():
            return psA.tile([128, SC], F32, tag="mm", name="pq")

        def mm(t, dc):
            nc.tensor.matmul(t, w1t[dc][:, fb * 128:(fb + 1) * 128],
                             xts(sc, dc), start=(dc == 0), stop=(dc == NDC - 1))

        def tail(t):
            ssl1 = slice(sc * SC, (sc + 1) * SC)
            if fb < 2:
                nc.vector.tensor_scalar_add(qk[fb][:, ssl1], t,
                                            b6[:, fb:fb + 1])
            else:
                ke, ko = kpad[2 * (fb - 2)], kpad[2 * (fb - 2) + 1]
                nc.vector.tensor_scalar_add(ke[0:64, ssl1], t[0:64, :],
                                            b6[0:64, fb:fb + 1])
                nc.vector.tensor_scalar_add(ko[64:128, ssl1], t[64:128, :],
                                            b6[64:128, fb:fb + 1])
        return alloc, mm, tail

    def v_spec(sc, sb):
        jc = sc * 4 + sb

        def alloc():
            return psA.tile([128, F_V], F32, tag="mm", name="pv")

        def mm(t, dc):
            nc.tensor.matmul(t, xts(sc, dc)[:, sb * 128:(sb + 1) * 128],
                             w1t[dc][:, F_QK:FT],
                             start=(dc == 0), stop=(dc == NDC - 1))

        def tail(t):
            for h in range(HPC):
                nc.vector.tensor_add(
                    v4[jc][:, h * (HD + 1):h * (HD + 1) + HD],
                    pv_bias(t, h), bv[:, h * HD:(h + 1) * HD])
                nc.vector.tensor_copy(
                    v4[jc][:, h * (HD + 1) + HD:(h + 1) * (HD + 1)], ones)

        def pv_bias(t, h):
            return t[:, h * HD:(h + 1) * HD]
        return alloc, mm, tail

    specs = []
    for sc in range(NSC):
        specs.append(qk_spec(sc, 2))
        specs.append(qk_spec(sc, 3))
        for sb in range(4):
            specs.append(v_spec(sc, sb))
    specs += [qk_spec(0, 0), qk_spec(1, 0), qk_spec(0, 1), qk_spec(1, 1)]

    prev = None       # (mm, tail, tile)
    for alloc, mm, tail in specs:
        t = alloc()
        for dc in range(4):
            if prev is not None:
                prev[0](prev[2], 4 + dc)
            mm(t, dc)
        if prev is not None:
            prev[1](prev[2])
        prev = (mm, tail, t)
    for dc in range(4, NDC):
        prev[0](prev[2], dc)
    prev[1](prev[2])

    workq.extend(q_steps(2, 0))                      # q pair0 i 1024-2047
    workq.extend(q_steps(3, 0))
    workq.extend(q_steps(2, 1))                      # q pair1 i 1024-2047
    workq.extend(q_steps(3, 1))

    # ---- attention stream ----
    # entries: (h-index, i0, width). The last processed head of the last
    # chunk is an even head (h=2: its normalize needs no partition-shift
    # DMA) and runs as two 512-wide passes so the final out-projection
    # overlaps its exp stream.
    entries = [(h, 0, 1024) for h in range(HPC)]
    entries += [(0, 1024, 1024), (1, 1024, 1024), (3, 1024, 1024),
                (2, 1024, 512), (2, 1536, 512)]

    def make_ss_half(ent, jc, c0):
        # 512-wide micro-steps with a 4-deep ss rotation (same 4 PSUM banks
        # as 2x1024): shorter stages double the pipeline depth and absorb
        # cross-engine semaphore latency between scores/exp/PV
        h, i0, width = ent
        ss = psS.tile([128, SC], F32, tag="ss", bufs=4, name="ss")
        nc.tensor.matmul(ss, kpad[h][:, jc * 128:(jc + 1) * 128],
                         qk[h // 2][:, i0 + c0:i0 + c0 + SC],
                         start=True, stop=True)
        return ss

    def normalize(h, i0, width, po):
        p = h // 2
        for c0 in range(0, width, SC):
            isl = slice(i0 + c0, i0 + c0 + SC)
            posb = spool.tile([HD + 1, SC], F32, tag="posb", name="posb")
            nc.vector.tensor_copy(posb, po[c0 // SC])
            recip = spool.tile([1, SC], F32, tag="recip", name="recip")
            nc.vector.reciprocal(recip, posb[HD:HD + 1, :])
            rb = spool.tile([HD, SC], F32, tag="rb", name="rb")
            nc.gpsimd.partition_broadcast(rb, recip)
            if h % 2 == 0:
                nc.vector.tensor_mul(outT[p][0:HD, isl], posb[0:HD, :], rb)
            else:
                tmp = spool.tile([HD, SC], F32R, tag="tmp64", name="tmp64")
                nc.vector.tensor_mul(tmp, posb[0:HD, :], rb)
                nc.sync.dma_start(out=outT[p][HD:128, isl], in_=tmp)

    msteps = []
    for idx, ent in enumerate(entries):
        for jc in range(NJ):
            for c0 in range(0, ent[2], SC):
                msteps.append((idx, jc, c0))

    po_by_idx = {}
    sstile = make_ss_half(entries[0], 0, 0)
    for mi, (idx, jc, c0) in enumerate(msteps):
        h, i0, width = entries[idx]
        if jc == 0 and c0 == 0:
            po_by_idx[idx] = [psO.tile([HD + 1, SC], F32, tag="po",
                                       name=f"po_{cc}")
                              for cc in range(0, width, SC)]
        ex = epool.tile([128, SC], BF16, tag="ex", bufs=12, name="ex")
        nc.scalar.activation(ex, sstile, AF.Exp, bias=0.0, scale=0.125)
        # last entry: hold the remaining projection steps back so they
        # bridge the final normalize (keeps PE busy through the tail)
        if idx < len(entries) - 1 and jc % 4 == 0 and c0 == 0 and jc < NJ - 2:
            drain(1)
        if mi + 1 < len(msteps):
            nidx, njc, nc0 = msteps[mi + 1]
            sstile = make_ss_half(entries[nidx], njc, nc0)
        nc.tensor.matmul(po_by_idx[idx][c0 // SC],
                         v4[jc][:, h * (HD + 1):(h + 1) * (HD + 1)], ex,
                         start=(jc == 0), stop=(jc == NJ - 1))
        if jc == NJ - 1 and c0 == width - SC:
            normalize(h, i0, width, po_by_idx.pop(idx))
            drain(2)
            # queue projection for s-rows whose outT columns just completed
            if idx == 3:
                for sblk in range(8):
                    workq.extend(make_proj_steps(sblk))
            elif idx == 7:
                for sblk in range(8, 12):
                    workq.extend(make_proj_steps(sblk, tail=True))
            elif idx == 8:
                for sblk in range(12, 16):
                    workq.extend(make_proj_steps(sblk, tail=True))
    while workq:
        workq.popleft()()


def _shards(x, W_qkv, b_qkv, W_proj):
    """Build per-core input maps."""
    import ml_dtypes
    bf16 = ml_dtypes.bfloat16
    xTb = [np.ascontiguousarray(x[b].T.astype(bf16)) for b in range(B)]
    in_maps = []
    for c in range(N_CORES):
        b, g = c // 4, c % 4
        cols = slice(g * HPC * HD, (g + 1) * HPC * HD)  # 256 cols within q/k/v
        W1 = np.concatenate([W_qkv[:, 0 * D:1 * D][:, cols],
                             W_qkv[:, 1 * D:2 * D][:, cols],
                             W_qkv[:, 2 * D:3 * D][:, cols]], axis=1)
        b1 = np.concatenate([b_qkv[0 * D:1 * D][cols],
                             b_qkv[1 * D:2 * D][cols],
                             b_qkv[2 * D:3 * D][cols]]).reshape(FT, 1)
        Wp = W_proj[g * HPC * HD:(g + 1) * HPC * HD, :]
        in_maps.append({
            "xT": xTb[b],
            "W1": np.ascontiguousarray(W1.astype(bf16)),
            "b1": np.ascontiguousarray(b1, dtype=np.float32),
            "Wp": np.ascontiguousarray(Wp, dtype=np.float32),
        })
    return in_maps


def kernel(x, W_qkv, b_qkv, W_proj, b_proj):
    from concourse.bass_utils import run_bass_kernel_spmd

    x = np.asarray(x, dtype=np.float32)
    W_qkv = np.asarray(W_qkv, dtype=np.float32)
    b_qkv = np.asarray(b_qkv, dtype=np.float32)
    W_proj = np.asarray(W_proj, dtype=np.float32)
    b_proj = np.asarray(b_proj, dtype=np.float32)

    if "nc" not in _CACHE:
        _CACHE["nc"] = _build()
    nc = _CACHE["nc"]

    in_maps = _shards(x, W_qkv, b_qkv, W_proj)
    res = run_bass_kernel_spmd(nc, in_maps, list(range(N_CORES)), trace=False)

    out = np.empty((B, S, D), dtype=np.float32)
    for b in range(B):
        acc = res.results[4 * b]["y"].astype(np.float32)
        for g in range(1, 4):
            acc = acc + res.results[4 * b + g]["y"].astype(np.float32)
        out[b] = acc + b_proj[None, :]
    return out


if __name__ == "__main__":
    rng = np.random.default_rng(0)
    scale = 1.0 / np.sqrt(D)
    inputs = {
        "x": rng.standard_normal((B, S, D), dtype=np.float32),
        "W_qkv": (rng.standard_normal((D, 3 * D)).astype(np.float32) * scale),
        "b_qkv": np.zeros(3 * D, np.float32),
        "W_proj": (rng.standard_normal((D, D)).astype(np.float32) * scale),
        "b_proj": np.zeros(D, np.float32),
    }
    out = kernel(**inputs)
    print("out", out.shape, out.dtype, np.abs(out).max())

